# revision 7
# baseline (speedup 1.0000x reference)
"""Trainium2 Bass kernel for nn_DiffusionPolicy (20-step diffusion sampler).

Self-contained: takes full unsharded inputs, shards batch rows across 8
NeuronCores (pure data parallel), runs a Bass/Tile kernel per core, gathers.

Layout: feature-major activations [feature_partition, row_free]. The state
conditioning (cond / modulation scale+shift) is computed once per core on
512 distinct states and broadcast to the 32 samples per state via step-0
access patterns. The 20 diffusion steps run as a hardware For loop; each
step sweeps 16 row-chunks of 1024 rows.
"""
import sys
sys.path.insert(0, '/opt/trn_rl_repo')
import math
import numpy as np

import concourse.bass as bass
import concourse.tile as tile
from concourse import mybir
from concourse.bass_utils import run_bass_kernel_spmd

F32 = mybir.dt.float32
F16 = mybir.dt.float16
AOP = mybir.AluOpType
AFT = mybir.ActivationFunctionType

NUM_STEPS = 20
NCORES = 8
BATCH = 4096
NSAMP = 32
N = BATCH * NSAMP            # 131072
R = N // NCORES              # 16384 rows per core
SB = BATCH // NCORES         # 512 states per core
FCH = 1024                   # rows per chunk
NCHUNK = R // FCH            # 16
SCH = FCH // NSAMP           # 32 states per chunk
NHID = 4

# per-layer home for the "+shift2" add: 'pe' (psum inject) or 'gp'
ADD_HOME = ['pe', 'pe', 'gp', 'gp']

_PROGRAM = None  # cached Bass program


def _split_wide_waits(nc, limit=1):
    """This walrus build rejects instructions carrying more than one sem
    wait. Hoist excess waits onto injected helper NoOps placed before the
    original instruction (same engine, so blocking semantics match)."""
    n_fixed = 0
    for f in nc.m.functions:
        for bb in f.blocks:
            insts = bb.instructions
            new_list = []
            changed = False
            for inst in insts:
                si = inst.sync_info
                w = list(si.on_wait) if si and si.on_wait else []
                if len(w) > limit:
                    extra, keep = w[:-limit], w[-limit:]
                    for wait in extra:
                        nm = nc.get_next_instruction_name()
                        h = mybir.InstNoOp(
                            name=nm, engine=inst.engine,
                            sync_info=mybir.SyncInfo(on_wait=[wait], on_update=[]),
                            ins=[], outs=[])
                        new_list.append(h)
                    si.on_wait = keep
                    inst.sync_info = si
                    changed = True
                    n_fixed += 1
                new_list.append(inst)
            if changed:
                bb.instructions = new_list
    return n_fixed


def _expand3(ap_small):
    """[128, S] -> [128, S, NSAMP] broadcast view."""
    p, s = ap_small.shape
    return ap_small.unsqueeze(2).broadcast_to([p, s, NSAMP])


def build_program(repeat=None):
    nc = bass.Bass()

    # ---------------- DRAM parameters (per core) ----------------
    dp = nc.declare_dram_parameter
    statesT_d = dp("statesT", [2, SB], F16, isOutput=False)
    wse1_d = dp("wse1", [2, 128], F16, isOutput=False)
    bse1_d = dp("bse1", [128, 1], F32, isOutput=False)
    wse2_d = dp("wse2", [128, 128], F16, isOutput=False)
    bse2_d = dp("bse2", [128, 1], F32, isOutput=False)
    wsc_d = dp("wsc", [128, NHID, 256], F16, isOutput=False)
    bsc1_d = dp("bsc1", [128, NHID * 2], F32, isOutput=False)   # 1 + b_scale, per (i, m)
    wsh_d = dp("wsh", [128, NHID, 256], F16, isOutput=False)
    bsh_d = dp("bsh", [128, NHID * 2], F32, isOutput=False)
    bhid_d = dp("bhid", [128, NHID * 2], F32, isOutput=False)
    whid_d = dp("whid", [128, NHID, 2, 2, 128], F16, isOutput=False)  # [k-row, i, kt, mt, mcol]
    ident_d = dp("ident", [128, 128], F16, isOutput=False)
    winA_d = dp("winA", [3, NUM_STEPS, 256], F16, isOutput=False)
    wout_d = dp("wout", [128, NUM_STEPS, 2, 2], F16, isOutput=False)  # [k-row, l, kt, m]
    mw_d = dp("mw", [3, NUM_STEPS, 3, 2], F16, isOutput=False)  # [:,l,0]=c1s*I2 [:,l,1]=c2pick [:,l,2]=I2
    xsc_d = dp("xsc", [2, NUM_STEPS, 4], F32, isOutput=False)   # cols: nkb, a, -a, 0
    x0T_d = dp("x0T", [3, R], F16, isOutput=False)   # rows 0-1: x0, row 2: ones
    snT_d = dp("snT", [2, NUM_STEPS, R], F16, isOutput=False)
    outT_d = dp("outT", [2, R], F32, isOutput=True)

    with tile.TileContext(nc) as tc:
        with (
            tc.tile_pool(name="persist", bufs=1) as pp,
            tc.tile_pool(name="stage", bufs=2) as stg,
            tc.tile_pool(name="work", bufs=2) as wk,
            tc.tile_pool(name="phaseA", bufs=1) as pa,
            tc.tile_pool(name="hpool", bufs=3) as hp,
            tc.tile_pool(name="gpool", bufs=2) as gb,
            tc.tile_pool(name="snp", bufs=1) as snp,
            tc.tile_pool(name="ph", bufs=3, space="PSUM") as php,
            tc.tile_pool(name="px", bufs=2, space="PSUM") as pxp,
        ):
            # ------------- load persistent weights -------------
            wse1 = pp.tile([2, 128], F16)
            wse2 = pp.tile([128, 128], F16)
            bse1 = pp.tile([128, 1], F32)
            bse2 = pp.tile([128, 1], F32)
            wsc = pp.tile([128, NHID, 256], F16)
            wsh = pp.tile([128, NHID, 256], F16)
            bsc1 = pp.tile([128, NHID * 2], F32)
            bsh = pp.tile([128, NHID * 2], F32)
            bhid = pp.tile([128, NHID * 2], F32)
            whid = pp.tile([128, NHID, 2, 2, 128], F16)
            ident = pp.tile([128, 128], F16)
            statesT = pp.tile([2, SB], F16)
            for t, d in [(wse1, wse1_d), (wse2, wse2_d), (bse1, bse1_d),
                         (bse2, bse2_d), (wsc, wsc_d), (wsh, wsh_d),
                         (bsc1, bsc1_d), (bsh, bsh_d), (bhid, bhid_d),
                         (whid, whid_d), (ident, ident_d), (statesT, statesT_d)]:
                nc.sync.dma_start(t[:], d[:])

            # persistent state across the step loop
            xa = pp.tile([3, R], F16)          # rows 0-1: x, row 2: ones
            s1p = pp.tile([128, NHID, 2, SB], F16)
            shift2 = pp.tile([128, NHID, 2, SB], F16)
            sh2ds = pp.tile([128, NHID, 2, SB], F16)

            nc.sync.dma_start(xa[:], x0T_d[:])

            # ------------- phase A: per-state conditioning -------------
            pA = pxp.tile([128, SB], F32, tag="px")
            nc.tensor.matmul(pA[:], wse1[:], statesT[:], start=True, stop=True)
            h1 = pa.tile([128, SB], F16, tag="h1")
            nc.scalar.activation(h1[:], pA[:], AFT.Silu, bias=bse1[:, 0:1], scale=1.0)
            pB = pxp.tile([128, SB], F32, tag="px")
            nc.tensor.matmul(pB[:], wse2[:], h1[:], start=True, stop=True)
            cond = pp.tile([128, SB], F16)
            nc.scalar.activation(cond[:], pB[:], AFT.Silu, bias=bse2[:, 0:1], scale=1.0)

            for i in range(NHID):
                for m in range(2):
                    col = i * 2 + m
                    psS = pxp.tile([128, SB], F32, tag="px")
                    nc.tensor.matmul(psS[:], wsc[:, i, m * 128:(m + 1) * 128], cond[:],
                                     start=True, stop=True)
                    s1f = pa.tile([128, SB], F32, tag="s1f")
                    nc.vector.tensor_scalar(s1f[:], psS[:], bsc1[:, col:col + 1], None, AOP.add)
                    nc.vector.tensor_copy(s1p[:, i, m, :], s1f[:])
                    psT = pxp.tile([128, SB], F32, tag="px")
                    nc.tensor.matmul(psT[:], wsh[:, i, m * 128:(m + 1) * 128], cond[:],
                                     start=True, stop=True)
                    s2a = pa.tile([128, SB], F32, tag="s2a")
                    nc.vector.tensor_scalar(s2a[:], psT[:], bsh[:, col:col + 1], None, AOP.add)
                    # shift2 = s1p*bhid + shift
                    nc.vector.scalar_tensor_tensor(shift2[:, i, m, :], s1f[:],
                                                   bhid[:, col:col + 1], s2a[:],
                                                   AOP.mult, AOP.add)
                    # shift2ds = bhid + shift / s1p
                    rec = pa.tile([128, SB], F32, tag="rec")
                    nc.vector.reciprocal(rec[:], s1f[:])
                    tmp = pa.tile([128, SB], F32, tag="tmp")
                    nc.vector.tensor_mul(tmp[:], s2a[:], rec[:])
                    nc.vector.tensor_scalar(sh2ds[:, i, m, :], tmp[:],
                                            bhid[:, col:col + 1], None, AOP.add)

            # ------------- phase B: 20 diffusion steps -------------
            import contextlib
            rep_ctx = (tc.For_i(0, repeat) if repeat else contextlib.nullcontext())
            with rep_ctx, tc.For_i(0, NUM_STEPS, hint_engines=(mybir.EngineType.PE,)) as l:
                winA = stg.tile([3, 256], F16, tag="winA")
                nc.sync.dma_start(winA[:].unsqueeze(1), winA_d[:, bass.ds(l, 1), :])
                wout = stg.tile([128, 2, 2], F16, tag="wout")
                nc.sync.dma_start(wout[:].unsqueeze(1), wout_d[:, bass.ds(l, 1), :, :])
                mw = stg.tile([3, 3, 2], F16, tag="mw")
                nc.sync.dma_start(mw[:].unsqueeze(1), mw_d[:, bass.ds(l, 1), :, :])
                xsc = stg.tile([2, 4], F32, tag="xsc")
                nc.sync.dma_start(xsc[:].unsqueeze(1), xsc_d[:, bass.ds(l, 1), :])
                sn = snp.tile([2, R], F16, tag="sn")
                for q in range(4):
                    nc.sync.dma_start(sn[:, q * (R // 4):(q + 1) * (R // 4)].unsqueeze(1),
                                      snT_d[:, bass.ds(l, 1), q * (R // 4):(q + 1) * (R // 4)])

                for c in range(NCHUNK):
                    xac = xa[:, c * FCH:(c + 1) * FCH]       # [3, 1024]
                    snc = sn[:, c * FCH:(c + 1) * FCH]
                    s0 = c * SCH

                    # ---- input layer: h0 = x_aug @ WinA ----
                    h = hp.tile([128, 2048], F16, tag="h")
                    for m in range(2):
                        ph0 = php.tile([128, 1024], F32, tag="ph")
                        for hh in range(2):
                            nc.tensor.matmul(
                                ph0[:, hh * 512:(hh + 1) * 512],
                                winA[:, m * 128:(m + 1) * 128],
                                xac[:, hh * 512:(hh + 1) * 512],
                                start=True, stop=True)
                        nc.scalar.activation(h[:, m * 1024:(m + 1) * 1024], ph0[:],
                                             AFT.Copy, bias=0.0, scale=1.0)

                    # ---- hidden layers ----
                    for i in range(NHID):
                        inject = ADD_HOME[i] == 'pe'
                        g = gb.tile([128, 2048], F16, tag="g")
                        for m in range(2):
                            ph = php.tile([128, 1024], F32, tag="ph")
                            for hh in range(2):
                                out_sl = ph[:, hh * 512:(hh + 1) * 512]
                                if inject:
                                    nc.tensor.matmul(
                                        out_sl, ident[:],
                                        _expand3(sh2ds[:, i, m, s0:s0 + SCH])
                                        [:, hh * 16:(hh + 1) * 16, :],
                                        start=True, stop=False)
                                for kt in range(2):
                                    nc.tensor.matmul(
                                        out_sl, whid[:, i, kt, m, :],
                                        h[:, kt * 1024 + hh * 512: kt * 1024 + (hh + 1) * 512],
                                        start=(kt == 0 and not inject),
                                        stop=(kt == 1))
                            nc.vector.tensor_mul(
                                g[:, m * 1024:(m + 1) * 1024].rearrange(
                                    "p (s r) -> p s r", r=NSAMP),
                                ph[:].rearrange("p (s r) -> p s r", r=NSAMP),
                                _expand3(s1p[:, i, m, s0:s0 + SCH]))
                        if inject:
                            u = g
                        else:
                            u = gb.tile([128, 2048], F16, tag="u")
                            nc.gpsimd.tensor_add(
                                u[:].rearrange("p (m s r) -> p m s r", m=2, r=NSAMP),
                                g[:].rearrange("p (m s r) -> p m s r", m=2, r=NSAMP),
                                shift2[:, i, :, s0:s0 + SCH].unsqueeze(3)
                                .broadcast_to([128, 2, SCH, NSAMP]))
                        h = hp.tile([128, 2048], F16, tag="h")
                        nc.scalar.activation(h[:], u[:], AFT.Silu)

                    # ---- x-path ----
                    t2 = wk.tile([2, FCH], F16, tag="t2")
                    for hh in range(2):
                        px = pxp.tile([128, 512], F32, tag="px")
                        nc.tensor.matmul(px[0:2, :], wout[:, 0, :],
                                         h[:, hh * 512:(hh + 1) * 512],
                                         start=True, stop=False)
                        nc.tensor.matmul(px[0:2, :], wout[:, 1, :],
                                         h[:, 1024 + hh * 512: 1024 + (hh + 1) * 512],
                                         start=False, stop=True)
                        # t2 = (eps_scaled + nkb) + x
                        nc.vector.scalar_tensor_tensor(
                            t2[:, hh * 512:(hh + 1) * 512], px[0:2, :],
                            xsc[:, 0:1], xac[0:2, hh * 512:(hh + 1) * 512],
                            AOP.add, AOP.add)
                    x0p = wk.tile([2, FCH], F16, tag="x0p")
                    nc.gpsimd.tensor_scalar(x0p[:], t2[:], xsc[:, 1:2], xsc[:, 2:3],
                                            AOP.min, AOP.max)
                    for hh in range(2):
                        pm = pxp.tile([128, 512], F32, tag="px")
                        nc.tensor.matmul(pm[0:2, :], mw[:, 1, :],
                                         xac[:, hh * 512:(hh + 1) * 512],
                                         start=True, stop=False)
                        nc.tensor.matmul(pm[0:2, :], mw[0:2, 2, :],
                                         snc[:, hh * 512:(hh + 1) * 512],
                                         start=False, stop=False)
                        nc.tensor.matmul(pm[0:2, :], mw[0:2, 0, :],
                                         x0p[:, hh * 512:(hh + 1) * 512],
                                         start=False, stop=True)
                        nc.vector.tensor_scalar(xac[0:2, hh * 512:(hh + 1) * 512],
                                                pm[0:2, :], 1.5, -1.5, AOP.min, AOP.max)

            # ------------- output -------------
            for c in range(NCHUNK):
                o32 = wk.tile([2, FCH], F32, tag="o32")
                nc.scalar.copy(o32[:], xa[0:2, c * FCH:(c + 1) * FCH])
                nc.sync.dma_start(outT_d[:, c * FCH:(c + 1) * FCH], o32[:])

    _split_wide_waits(nc)
    return nc


# ---------------------------------------------------------------------------
# host side
# ---------------------------------------------------------------------------

def host_precompute(inputs):
    import jax
    import jax.numpy as jnp

    cpu = jax.devices("cpu")[0]
    f16 = np.float16

    W_in = np.asarray(inputs['W_in'], np.float32)
    b_in = np.asarray(inputs['b_in'], np.float32)
    W_out = np.asarray(inputs['W_out'], np.float32)
    b_out = np.asarray(inputs['b_out'], np.float32)
    W_hid = np.asarray(inputs['W_hid'], np.float32)
    b_hid = np.asarray(inputs['b_hid'], np.float32)
    b_scale = np.asarray(inputs['b_scale'], np.float32)
    b_shift = np.asarray(inputs['b_shift'], np.float32)
    W_scale = np.asarray(inputs['W_scale'], np.float32)
    W_shift = np.asarray(inputs['W_shift'], np.float32)
    states = np.asarray(inputs['states'], np.float32)

    with jax.default_device(cpu):
        betas = jnp.linspace(1e-4, 0.02, NUM_STEPS, dtype=jnp.float32)
        alphas = 1.0 - betas
        ac = jnp.cumprod(alphas)
        ac_prev = jnp.concatenate([jnp.ones((1,), jnp.float32), ac[:-1]])
        one_m = jnp.maximum(1.0 - ac, 1e-8)
        src = np.asarray(jnp.sqrt(1.0 / ac))
        srm1 = np.asarray(jnp.sqrt(1.0 / ac - 1.0))
        c1 = np.asarray(betas * jnp.sqrt(ac_prev) / one_m)
        c2 = np.asarray((1.0 - ac_prev) * jnp.sqrt(alphas) / one_m)
        plv = np.asarray(jnp.log(jnp.maximum(betas * (1.0 - ac_prev) / one_m, 1e-20)))
        freqs = np.asarray(jnp.exp(jnp.linspace(0.0, math.log(1000.0), 16)))
        kx, kn = jax.random.split(jax.random.key(42))
        x0 = np.asarray(jax.random.normal(kx, (N, 2), dtype=jnp.float32))
        noises = np.asarray(jax.random.normal(kn, (NUM_STEPS, N, 2), dtype=jnp.float32))

    sigma = np.where(np.arange(NUM_STEPS) > 0, np.exp(0.5 * plv), 0.0).astype(np.float32)
    sn_full = (sigma[:, None, None] * noises).astype(np.float32)   # [t, N, 2]

    # loop order l = 0..19  <->  diffusion t = 19-l
    ts = np.arange(NUM_STEPS - 1, -1, -1)

    winA = np.zeros((3, NUM_STEPS, 256), np.float32)
    wout = np.zeros((128, NUM_STEPS, 2, 2), np.float32)
    mw = np.zeros((3, NUM_STEPS, 3, 2), np.float32)
    xsc = np.zeros((2, NUM_STEPS, 4), np.float32)
    for li, t in enumerate(ts):
        ang = np.float32(t) * freqs
        emb = np.concatenate([np.sin(ang), np.cos(ang)]).astype(np.float32)
        winA[0:2, li] = W_in[0:2]
        winA[2, li] = emb @ W_in[2:34] + b_in
        k_t = srm1[t] / src[t]
        ws = (-k_t * W_out).reshape(2, 128, 2)                  # [kt, krow, m]
        wout[:, li] = ws.transpose(1, 0, 2)
        c1s = c1[t] * src[t]
        mw[0, li, 0] = [c1s, 0.0]
        mw[1, li, 0] = [0.0, c1s]
        mw[0, li, 1] = [c2[t], 0.0]
        mw[1, li, 1] = [0.0, c2[t]]
        mw[0, li, 2] = [1.0, 0.0]
        mw[1, li, 2] = [0.0, 1.0]
        xsc[:, li, 0] = -k_t * b_out
        xsc[:, li, 1] = 1.0 / src[t]
        xsc[:, li, 2] = -1.0 / src[t]

    whid_t = np.zeros((128, NHID, 2, 2, 128), np.float32)
    for i in range(NHID):
        w4 = W_hid[i].reshape(2, 128, 2, 128)                   # [kt, krow, mt, mcol]
        whid_t[:, i] = w4.transpose(1, 0, 2, 3)

    snT_l = sn_full[ts].transpose(2, 0, 1)                      # [2, 20, N] loop order

    shared = {
        "wse1": np.asarray(inputs['W_se1'], f16),
        "bse1": np.asarray(inputs['b_se1'], np.float32).reshape(128, 1),
        "wse2": np.asarray(inputs['W_se2'], f16),
        "bse2": np.asarray(inputs['b_se2'], np.float32).reshape(128, 1),
        "wsc": W_scale.transpose(1, 0, 2).astype(f16),
        "bsc1": np.ascontiguousarray(
            (1.0 + b_scale).reshape(NHID, 2, 128).transpose(2, 0, 1)).reshape(128, NHID * 2),
        "wsh": W_shift.transpose(1, 0, 2).astype(f16),
        "bsh": np.ascontiguousarray(
            b_shift.reshape(NHID, 2, 128).transpose(2, 0, 1)).reshape(128, NHID * 2),
        "bhid": np.ascontiguousarray(
            b_hid.reshape(NHID, 2, 128).transpose(2, 0, 1)).reshape(128, NHID * 2),
        "whid": whid_t.astype(f16),
        "ident": np.eye(128, dtype=f16),
        "winA": winA.astype(f16),
        "wout": wout.astype(f16),
        "mw": mw.astype(f16),
        "xsc": xsc,
    }
    per_core = []
    for cidx in range(NCORES):
        srows = slice(cidx * R, (cidx + 1) * R)
        sstates = slice(cidx * SB, (cidx + 1) * SB)
        m = dict(shared)
        m["statesT"] = np.ascontiguousarray(states[sstates].T).astype(f16)
        m["x0T"] = np.concatenate([np.ascontiguousarray(x0[srows].T),
                                   np.ones((1, R), np.float32)], axis=0).astype(f16)
        m["snT"] = np.ascontiguousarray(snT_l[:, :, srows]).astype(f16)
        per_core.append(m)
    return per_core


def kernel(**inputs):
    global _PROGRAM
    if _PROGRAM is None:
        _PROGRAM = build_program()
    nc = _PROGRAM
    per_core = host_precompute(inputs)
    res = run_bass_kernel_spmd(nc, per_core, list(range(NCORES)))
    outs = [res.results[c]["outT"] for c in range(NCORES)]       # [2, R] each
    full = np.concatenate([o.T for o in outs], axis=0)           # [N, 2]
    return np.ascontiguousarray(full.reshape(BATCH, NSAMP, 2), dtype=np.float32)
